# revision 1
# baseline (speedup 1.0000x reference)
"""Depthwise cross-correlation (pysot xcorr_depthwise) on 8 Trainium2 NeuronCores.

Problem: kernel (B=128, C=256, 8, 8) x search (B=128, C=256, 32, 32)
-> out (B, C, 25, 25): per-(b,c) 2D valid cross-correlation, 32768 groups.

Strategy (pure data-parallel over batch; 16 batches = 4096 groups per core):
  - Groups are processed in "sets" of 16. One DMA per set loads an 8-way
    element-shifted replica tile REP[(g,v), 0:1024] = S_flat[g, v : v+1024]
    (v = horizontal tap; row padding makes the tail reads safe), so the
    horizontal shifts live on partitions and the vertical shifts in the
    free-dim access pattern.
  - The stationary operand is a host-built block-diagonal weight matrix
    LD[(g,v), (u,g')] = delta(g,g') * K[g,u,v].
  - 8 accumulating bf16 matmuls (one per kernel row u) read REP at row
    offset u, so PSUM accumulation performs the vertical tap sum:
      psum[g, (i,j)] += sum_v K[g,u,v] * S[g, i+u, j+v]
  - Output rows [0,13) accumulate in one PSUM bank (325 cols), rows
    [13,25) in the next (300 cols). DVE evacuates PSUM -> SBUF, GPSIMD
    SWDGE DMAs SBUF -> HBM.
  - Raw bass with hand-rolled semaphores; each engine runs a static
    unrolled program. 4 PSUM column-groups rotate across sets.
"""

import numpy as np

import concourse.bass as bass
import concourse.mybir as mybir
from concourse import bass_utils
from concourse.bass_types import AP

F32 = mybir.dt.float32
BF16 = mybir.dt.bfloat16

B, C, HK, WK, HS, WS = 128, 256, 8, 8, 32, 32
HO, WO = HS - HK + 1, WS - WK + 1  # 25, 25
N_CORES = 8
GSET = 16  # groups per set
SROW = HS * WS + WK  # padded flattened search row (1032)
OUT_F = HO * WO  # 625
SPLIT_I = 13  # output rows [0,13) -> first psum bank, [13,25) -> second
NA = SPLIT_I * WO  # 325
NB = (HO - SPLIT_I) * WO  # 300
REP_BUFS = 3
OUT_BUFS = 6
LD_CHUNK = 32  # sets per weight-preload DMA


def build_nc(n_sets: int, n_cores: int, passes: int = 1):
    nc = bass.Bass("TRN2", target_bir_lowering=False, debug=False, num_devices=n_cores)
    ng = n_sets * GSET
    s_in = nc.dram_tensor("s_in", [ng, SROW], BF16, kind="ExternalInput")
    ld_in = nc.dram_tensor("ld_in", [n_sets, 128, 128], BF16, kind="ExternalInput")
    out = nc.dram_tensor("out", [ng, OUT_F], F32, kind="ExternalOutput")

    chunk = min(LD_CHUNK, n_sets)
    n_chunks = (n_sets + chunk - 1) // chunk

    ld_full = nc.alloc_sbuf_tensor("ld_full", [128, n_sets * 128], BF16)
    rep_bufs = [
        nc.alloc_sbuf_tensor(f"rep{i}", [128, 1024], BF16) for i in range(REP_BUFS)
    ]
    out_bufs = [
        nc.alloc_sbuf_tensor(f"osb{i}", [128, OUT_F], F32) for i in range(OUT_BUFS)
    ]
    ps = nc.alloc_psum_tensor("ps", [128, 4096], F32)

    from contextlib import ExitStack

    with ExitStack() as ctx:
        block = ctx.enter_context(nc.Block())
        s_ld = [
            ctx.enter_context(nc.semaphore(f"s_ld{i}")) for i in range(n_chunks)
        ]
        s_rep = [
            ctx.enter_context(nc.semaphore(f"s_rep{i}")) for i in range(REP_BUFS)
        ]
        s_pe = ctx.enter_context(nc.semaphore("s_pe"))
        s_dve = ctx.enter_context(nc.semaphore("s_dve"))
        s_out = [
            ctx.enter_context(nc.semaphore(f"s_out{i}")) for i in range(OUT_BUFS)
        ]

        @block.sync
        def _(sync):
            # Weight preload: LD[s] matrices laid side by side in SBUF.
            for ci in range(n_chunks):
                c0 = ci * chunk
                nsc = min(chunk, n_sets - c0)
                sync.dma_start(
                    ld_full.ap()[:, c0 * 128 : (c0 + nsc) * 128],
                    AP(ld_in, c0 * 128 * 128, [[128, 128], [128 * 128, nsc], [1, 128]]),
                ).then_inc(s_ld[ci], 16)
            # Replica loads, triple buffered.
            for t in range(passes * n_sets):
                s = t % n_sets
                if t >= REP_BUFS:
                    sync.wait_ge(s_pe, t - REP_BUFS + 1)
                sync.dma_start(
                    rep_bufs[t % REP_BUFS].ap(),
                    AP(s_in, s * GSET * SROW, [[SROW, GSET], [1, WK], [1, 1024]]),
                ).then_inc(s_rep[t % REP_BUFS], 16)

        @block.tensor
        def _(tensor):
            for t in range(passes * n_sets):
                s = t % n_sets
                c = t % 4
                base, po = 32 * c, 1024 * c
                if t < n_sets and s % chunk == 0:
                    tensor.wait_ge(s_ld[s // chunk], 16)
                tensor.wait_ge(s_rep[t % REP_BUFS], 16 * (t // REP_BUFS + 1))
                if t >= 4:
                    tensor.wait_ge(s_dve, t - 3)  # psum col-group free
                rep = rep_bufs[t % REP_BUFS].ap()
                view = rep.rearrange("p (y x) -> p y x", x=WS)
                for u in range(HK):
                    lhsT = ld_full.ap()[
                        :, s * 128 + u * GSET : s * 128 + (u + 1) * GSET
                    ]
                    tensor.matmul(
                        ps.ap()[base : base + GSET, po : po + NA],
                        lhsT=lhsT,
                        rhs=view[:, u : u + SPLIT_I, 0:WO],
                        start=(u == 0),
                        stop=(u == HK - 1),
                        tile_position=(0, base),
                    )
                    mm = tensor.matmul(
                        ps.ap()[base : base + GSET, po + 512 : po + 512 + NB],
                        lhsT=lhsT,
                        rhs=view[:, u + SPLIT_I : u + HO, 0:WO],
                        start=(u == 0),
                        stop=(u == HK - 1),
                        tile_position=(0, base),
                    )
                mm.then_inc(s_pe, 1)

        @block.vector
        def _(vector):
            for t in range(passes * n_sets):
                c = t % 4
                base, po = 32 * c, 1024 * c
                vector.wait_ge(s_pe, t + 1)
                if t >= OUT_BUFS:
                    vector.wait_ge(s_out[t % OUT_BUFS], 16 * (t // OUT_BUFS))
                ob = out_bufs[t % OUT_BUFS].ap()
                vector.tensor_copy(
                    ob[base : base + GSET, 0:NA],
                    ps.ap()[base : base + GSET, po : po + NA],
                )
                vector.tensor_copy(
                    ob[base : base + GSET, NA:OUT_F],
                    ps.ap()[base : base + GSET, po + 512 : po + 512 + NB],
                ).then_inc(s_dve, 1)

        @block.gpsimd
        def _(gpsimd):
            nt = passes * n_sets
            for t in range(nt):
                s = t % n_sets
                base = 32 * (t % 4)
                gpsimd.wait_ge(s_dve, t + 1)
                gpsimd.dma_start(
                    AP(out, s * GSET * OUT_F, [[OUT_F, GSET], [1, OUT_F]]),
                    out_bufs[t % OUT_BUFS].ap()[base : base + GSET, :],
                ).then_inc(s_out[t % OUT_BUFS], 16)
            for i in range(OUT_BUFS):
                ns_i = (nt - 1 - i) // OUT_BUFS + 1 if i < nt else 0
                if ns_i > 0:
                    gpsimd.wait_ge(s_out[i], 16 * ns_i)

    return nc


def _host_prep(kernel: np.ndarray, search: np.ndarray, n_cores: int):
    """Per-core input maps: padded flat bf16 search + block-diag bf16 weights."""
    import ml_dtypes

    bf16 = ml_dtypes.bfloat16
    kernel = np.asarray(kernel, dtype=np.float32)
    search = np.asarray(search, dtype=np.float32)
    b, ch = kernel.shape[0], kernel.shape[1]
    bpc = b // n_cores
    g = bpc * ch
    n_sets = g // GSET
    eye = np.eye(GSET, dtype=np.float32)
    in_maps = []
    for c in range(n_cores):
        s_c = search[c * bpc : (c + 1) * bpc].reshape(g, HS * WS)
        s_pad = np.zeros((g, SROW), dtype=bf16)
        s_pad[:, : HS * WS] = s_c.astype(bf16)
        k5 = kernel[c * bpc : (c + 1) * bpc].reshape(n_sets, GSET, HK, WK)
        # LD[s, (g,v), (u,e)] = delta(g,e) * K[s,g,u,v]
        ld = np.einsum("sguv,ge->sgvue", k5, eye).reshape(n_sets, 128, 128)
        in_maps.append({"s_in": s_pad, "ld_in": np.ascontiguousarray(ld.astype(bf16))})
    return in_maps, n_sets


_CACHED = {}


def run(
    kernel: np.ndarray,
    search: np.ndarray,
    n_cores: int = N_CORES,
    passes: int = 1,
    **kw,
):
    in_maps, n_sets = _host_prep(kernel, search, n_cores)
    key = (n_sets, n_cores, passes)
    if key not in _CACHED:
        _CACHED[key] = build_nc(n_sets, n_cores, passes)
    nc = _CACHED[key]
    try:
        res = bass_utils.run_bass_kernel_spmd(
            nc, in_maps, core_ids=list(range(n_cores)), **kw
        )
    except (ModuleNotFoundError, ImportError):
        kw.pop("trace", None)
        res = bass_utils.run_bass_kernel_spmd(
            nc, in_maps, core_ids=list(range(n_cores)), **kw
        )
    b, ch = kernel.shape[0], kernel.shape[1]
    full = np.stack([r["out"] for r in res.results])  # [n_cores, g, 625]
    full = full.reshape(b, ch, HO, WO)
    return full, res


def kernel(**inputs) -> np.ndarray:
    out, _ = run(inputs["kernel"], inputs["search"])
    return out.astype(np.float32)



# revision 2
# speedup vs baseline: 1.3280x; 1.3280x over previous
"""Depthwise cross-correlation (pysot xcorr_depthwise) on 8 Trainium2 NeuronCores.

Problem: kernel (B=128, C=256, 8, 8) x search (B=128, C=256, 32, 32)
-> out (B, C, 25, 25): per-(b,c) 2D valid cross-correlation, 32768 groups.

Strategy (pure data-parallel over batch; 16 batches = 4096 groups per core):
  - Groups are processed in "sets" of 16. One DMA per set loads an 8-way
    element-shifted replica tile REP[(g,v), 0:1024] = S_flat[g, v : v+1024]
    (v = horizontal tap; row padding makes the tail reads safe), so the
    horizontal shifts live on partitions and the vertical shifts in the
    free-dim access pattern.
  - The stationary operand is a host-built block-diagonal weight matrix
    LD[(g,v), (u,g')] = delta(g,g') * K[g,u,v].
  - 8 accumulating bf16 matmuls (one per kernel row u) read REP at row
    offset u, so PSUM accumulation performs the vertical tap sum:
      psum[g, (i,j)] += sum_v K[g,u,v] * S[g, i+u, j+v]
  - Output rows [0,13) accumulate in one PSUM bank (325 cols), rows
    [13,25) in the next (300 cols). DVE evacuates PSUM -> SBUF, GPSIMD
    SWDGE DMAs SBUF -> HBM.
  - Raw bass with hand-rolled semaphores; each engine runs a static
    unrolled program. 4 PSUM column-groups rotate across sets.
"""

import numpy as np

import concourse.bass as bass
import concourse.mybir as mybir
from concourse import bass_utils
from concourse.bass_types import AP

F32 = mybir.dt.float32
BF16 = mybir.dt.bfloat16

B, C, HK, WK, HS, WS = 128, 256, 8, 8, 32, 32
HO, WO = HS - HK + 1, WS - WK + 1  # 25, 25
N_CORES = 8
GSET = 16  # groups per set
SROW = HS * WS + WK  # padded flattened search row (1032)
OUT_F = HO * WO  # 625
SPLIT_I = 13  # output rows [0,13) -> first psum bank, [13,25) -> second
NA = SPLIT_I * WO  # 325
NB = (HO - SPLIT_I) * WO  # 300
REP_BUFS = 4
OUT_BUFS = 8
LD_CHUNK = 16  # sets per weight-preload DMA


def build_nc(n_sets: int, n_cores: int, passes: int = 1):
    nc = bass.Bass("TRN2", target_bir_lowering=False, debug=False, num_devices=n_cores)
    ng = n_sets * GSET
    s_in = nc.dram_tensor("s_in", [ng, SROW], BF16, kind="ExternalInput")
    ld_in = nc.dram_tensor("ld_in", [n_sets, 128, 128], BF16, kind="ExternalInput")
    out = nc.dram_tensor("out", [ng, OUT_F], BF16, kind="ExternalOutput")

    chunk = min(LD_CHUNK, n_sets)
    n_chunks = (n_sets + chunk - 1) // chunk

    ld_full = nc.alloc_sbuf_tensor("ld_full", [128, n_sets * 128], BF16)
    rep_bufs = [
        nc.alloc_sbuf_tensor(f"rep{i}", [128, 1024], BF16) for i in range(REP_BUFS)
    ]
    out_bufs = [
        nc.alloc_sbuf_tensor(f"osb{i}", [128, OUT_F], BF16) for i in range(OUT_BUFS)
    ]
    ps = nc.alloc_psum_tensor("ps", [128, 4096], F32)

    from contextlib import ExitStack

    with ExitStack() as ctx:
        block = ctx.enter_context(nc.Block())
        s_ld = [
            ctx.enter_context(nc.semaphore(f"s_ld{i}")) for i in range(n_chunks)
        ]
        s_rep = [
            ctx.enter_context(nc.semaphore(f"s_rep{i}")) for i in range(REP_BUFS)
        ]
        s_pe = ctx.enter_context(nc.semaphore("s_pe"))
        s_dve = ctx.enter_context(nc.semaphore("s_dve"))
        s_out = [
            ctx.enter_context(nc.semaphore(f"s_out{i}")) for i in range(OUT_BUFS)
        ]

        @block.sync
        def _(sync):
            # Weight preload: LD[s] matrices laid side by side in SBUF.
            for ci in range(n_chunks):
                c0 = ci * chunk
                nsc = min(chunk, n_sets - c0)
                sync.dma_start(
                    ld_full.ap()[:, c0 * 128 : (c0 + nsc) * 128],
                    AP(ld_in, c0 * 128 * 128, [[128, 128], [128 * 128, nsc], [1, 128]]),
                ).then_inc(s_ld[ci], 16)
            # Replica loads, triple buffered.
            for t in range(passes * n_sets):
                s = t % n_sets
                if t >= REP_BUFS:
                    sync.wait_ge(s_pe, t - REP_BUFS + 1)
                sync.dma_start(
                    rep_bufs[t % REP_BUFS].ap(),
                    AP(s_in, s * GSET * SROW, [[SROW, GSET], [1, WK], [1, 1024]]),
                ).then_inc(s_rep[t % REP_BUFS], 16)

        @block.tensor
        def _(tensor):
            for t in range(passes * n_sets):
                s = t % n_sets
                c = t % 4
                base, po = 32 * c, 1024 * c
                if t < n_sets and s % chunk == 0:
                    tensor.wait_ge(s_ld[s // chunk], 16)
                tensor.wait_ge(s_rep[t % REP_BUFS], 16 * (t // REP_BUFS + 1))
                if t >= 4:
                    tensor.wait_ge(s_dve, t - 3)  # psum col-group free
                rep = rep_bufs[t % REP_BUFS].ap()
                view = rep.rearrange("p (y x) -> p y x", x=WS)
                for u in range(HK):
                    lhsT = ld_full.ap()[
                        :, s * 128 + u * GSET : s * 128 + (u + 1) * GSET
                    ]
                    tensor.matmul(
                        ps.ap()[base : base + GSET, po : po + NA],
                        lhsT=lhsT,
                        rhs=view[:, u : u + SPLIT_I, 0:WO],
                        start=(u == 0),
                        stop=(u == HK - 1),
                        tile_position=(0, base),
                    )
                    mm = tensor.matmul(
                        ps.ap()[base : base + GSET, po + 512 : po + 512 + NB],
                        lhsT=lhsT,
                        rhs=view[:, u + SPLIT_I : u + HO, 0:WO],
                        start=(u == 0),
                        stop=(u == HK - 1),
                        tile_position=(0, base),
                    )
                mm.then_inc(s_pe, 1)

        @block.vector
        def _(vector):
            for t in range(passes * n_sets):
                c = t % 4
                base, po = 32 * c, 1024 * c
                vector.wait_ge(s_pe, t + 1)
                if t >= OUT_BUFS:
                    vector.wait_ge(s_out[t % OUT_BUFS], 16 * (t // OUT_BUFS))
                ob = out_bufs[t % OUT_BUFS].ap()
                vector.tensor_copy(
                    ob[base : base + GSET, 0:NA],
                    ps.ap()[base : base + GSET, po : po + NA],
                )
                vector.tensor_copy(
                    ob[base : base + GSET, NA:OUT_F],
                    ps.ap()[base : base + GSET, po + 512 : po + 512 + NB],
                ).then_inc(s_dve, 1)

        @block.gpsimd
        def _(gpsimd):
            nt = passes * n_sets
            for t in range(nt):
                s = t % n_sets
                base = 32 * (t % 4)
                gpsimd.wait_ge(s_dve, t + 1)
                gpsimd.dma_start(
                    AP(out, s * GSET * OUT_F, [[OUT_F, GSET], [1, OUT_F]]),
                    out_bufs[t % OUT_BUFS].ap()[base : base + GSET, :],
                ).then_inc(s_out[t % OUT_BUFS], 16)
            for i in range(OUT_BUFS):
                ns_i = (nt - 1 - i) // OUT_BUFS + 1 if i < nt else 0
                if ns_i > 0:
                    gpsimd.wait_ge(s_out[i], 16 * ns_i)

    return nc


def _host_prep(kernel: np.ndarray, search: np.ndarray, n_cores: int):
    """Per-core input maps: padded flat bf16 search + block-diag bf16 weights."""
    import ml_dtypes

    bf16 = ml_dtypes.bfloat16
    kernel = np.asarray(kernel, dtype=np.float32)
    search = np.asarray(search, dtype=np.float32)
    b, ch = kernel.shape[0], kernel.shape[1]
    bpc = b // n_cores
    g = bpc * ch
    n_sets = g // GSET
    eye = np.eye(GSET, dtype=np.float32)
    in_maps = []
    for c in range(n_cores):
        s_c = search[c * bpc : (c + 1) * bpc].reshape(g, HS * WS)
        s_pad = np.zeros((g, SROW), dtype=bf16)
        s_pad[:, : HS * WS] = s_c.astype(bf16)
        k5 = kernel[c * bpc : (c + 1) * bpc].reshape(n_sets, GSET, HK, WK)
        # LD[s, (g,v), (u,e)] = delta(g,e) * K[s,g,u,v]
        ld = np.einsum("sguv,ge->sgvue", k5, eye).reshape(n_sets, 128, 128)
        in_maps.append({"s_in": s_pad, "ld_in": np.ascontiguousarray(ld.astype(bf16))})
    return in_maps, n_sets


_CACHED = {}


def run(
    kernel: np.ndarray,
    search: np.ndarray,
    n_cores: int = N_CORES,
    passes: int = 1,
    **kw,
):
    in_maps, n_sets = _host_prep(kernel, search, n_cores)
    key = (n_sets, n_cores, passes)
    if key not in _CACHED:
        _CACHED[key] = build_nc(n_sets, n_cores, passes)
    nc = _CACHED[key]
    try:
        res = bass_utils.run_bass_kernel_spmd(
            nc, in_maps, core_ids=list(range(n_cores)), **kw
        )
    except (ModuleNotFoundError, ImportError):
        kw.pop("trace", None)
        res = bass_utils.run_bass_kernel_spmd(
            nc, in_maps, core_ids=list(range(n_cores)), **kw
        )
    b, ch = kernel.shape[0], kernel.shape[1]
    full = np.stack([np.asarray(r["out"], dtype=np.float32) for r in res.results])
    full = full.reshape(b, ch, HO, WO)
    return full, res


def kernel(**inputs) -> np.ndarray:
    out, _ = run(inputs["kernel"], inputs["search"])
    return out.astype(np.float32)

